# revision 4
# baseline (speedup 1.0000x reference)
"""Trainium2 Bass kernel for nn_CrossAttention (B=32, C=256, H=W=32).

Data-parallel over 8 NeuronCores: core c processes batches 4c..4c+4.

Per-batch dataflow on one core (channel dim on partitions, HW on free):
  q  = Wq @ x1 + bq                      (fp16 matmuls, fp32 psum)
  k  = Wk1 @ x2 + (pool-term) + bk       pool-term via a [21 x HW] selector
                                         matmul (pyramid pools never upsampled)
  vT = x2^T @ Wv^T (+ ones column)       (bf16)
  S^T = k^T q                            computed directly in [j, i] layout
  P^T = exp(S^T)                         (bf16; no max-subtraction -- logits
                                          bounded, verified on actual data)
  outT[i, 257] = sum_j P^T[j,i] vT_ext[j,:]   -> col 256 = softmax denominator
  out = transpose(outT * 1/denom) + bv + x1   (PE transpose, fused DVE epilogue)
"""
import numpy as np
import ml_dtypes

import jax
from jax.sharding import Mesh, PartitionSpec
from jax.experimental.shard_map import shard_map

import concourse.bass as bass
import concourse.mybir as mybir
import concourse.tile as tile
from concourse.bass import ds
from concourse.bass2jax import _bass_exec_p, install_neuronx_cc_hook, partition_id_tensor

F32, F16, BF16, I32 = (mybir.dt.float32, mybir.dt.float16,
                       mybir.dt.bfloat16, mybir.dt.int32)
NCORES = 8
BPC = 4          # batches per core
HW = 1024
EXP = mybir.ActivationFunctionType.Exp
ADD = mybir.AluOpType.add
import os as _os
_PHASE = _os.environ.get("KERNEL_PHASE", "full")   # dma|proj|attn|full
_LOOP_MODE = _os.environ.get("KERNEL_LOOP", "plain")  # plain|staggered|hints


# ---------------------------------------------------------------- toolchain fix
def _split_excess_waits(nc, max_waits=1):
    """This walrus build rejects >1 sem wait per instruction ("Too many sync
    wait commands"); move excess waits onto preceding same-engine NOPs (the
    sequencer executes them in order, so semantics are preserved)."""
    n_split = 0
    for f in nc.m.functions:
        for bb in f.blocks:
            idx = 0
            while idx < len(bb.instructions):
                inst = bb.instructions[idx]
                si = inst.sync_info
                if si is not None and si.on_wait and len(si.on_wait) > max_waits:
                    waits = list(si.on_wait)
                    extra, keep = waits[:-max_waits], waits[-max_waits:]
                    pos = idx
                    for j in range(0, len(extra), max_waits):
                        chunk = extra[j:j + max_waits]
                        nop = mybir.InstNoOp(name=f"waitsplit-{n_split}", ins=[], outs=[])
                        n_split += 1
                        nop.engine = inst.engine
                        nop.sync_info = mybir.SyncInfo(on_wait=chunk, on_update=[])
                        nc.register_instruction(nop, overwrite=True)
                        bb.instructions.insert(pos, nop)
                        pos += 1
                    inst.sync_info = mybir.SyncInfo(
                        on_wait=keep, on_update=list(si.on_update or []))
                    idx = pos + 1
                else:
                    idx += 1
    return n_split


# ---------------------------------------------------------------- bass builder
def build_nc():
    nc = bass.Bass("TRN2")

    x1f_d = nc.dram_tensor("x1f", [BPC, 128, 2, HW], F32, kind="ExternalInput")
    x1h_d = nc.dram_tensor("x1h", [BPC, 128, 2, HW], F16, kind="ExternalInput")
    x2h_d = nc.dram_tensor("x2h", [BPC, 128, 2, HW], F16, kind="ExternalInput")
    x2b_d = nc.dram_tensor("x2b", [BPC, 128, 2, HW], BF16, kind="ExternalInput")
    wqt_d = nc.dram_tensor("wqt", [128, 2, 128], F16, kind="ExternalInput")
    wk1t_d = nc.dram_tensor("wk1t", [128, 2, 128], F16, kind="ExternalInput")
    wpool_d = nc.dram_tensor("wpool", [128, 6, 128], F16, kind="ExternalInput")
    wvt_d = nc.dram_tensor("wvt", [128, 2, 256], BF16, kind="ExternalInput")
    ssel_d = nc.dram_tensor("ssel", [21, HW], F16, kind="ExternalInput")
    ident_d = nc.dram_tensor("ident", [128, 128], F16, kind="ExternalInput")
    bq_d = nc.dram_tensor("bq", [128, 1], F32, kind="ExternalInput")
    bk_d = nc.dram_tensor("bk", [128, 1], F32, kind="ExternalInput")
    bv_d = nc.dram_tensor("bv", [128, 2, 1], F32, kind="ExternalInput")
    iters_d = nc.dram_tensor("iters", [1, 1], I32, kind="ExternalInput")
    out_d = nc.dram_tensor("out", [BPC, 128, 2, HW], F32, kind="ExternalOutput")

    with tile.TileContext(nc) as tc:
        with (
            tc.tile_pool(name="consts", bufs=1) as consts,
            tc.tile_pool(name="xin", bufs=2) as xin,
            tc.tile_pool(name="proj", bufs=2) as proj,
            tc.tile_pool(name="vtp", bufs=2) as vtp,
            tc.tile_pool(name="ptp", bufs=2) as ptp,
            tc.tile_pool(name="small", bufs=2) as small,
            tc.tile_pool(name="outp", bufs=2) as outp,
            tc.tile_pool(name="recipp", bufs=4) as recipp,
            tc.tile_pool(name="onp", bufs=3) as onp,
            tc.tile_pool(name="mm", bufs=2, space="PSUM") as mm,
            tc.tile_pool(name="vtps", bufs=2, space="PSUM") as vtps,
            tc.tile_pool(name="ops", bufs=4, space="PSUM") as ops_,
            nc.allow_low_precision("f16/bf16 intermediates by design"),
        ):
            wqt_sb = consts.tile([128, 2, 128], F16, tag="wqt")
            nc.sync.dma_start(wqt_sb[:], wqt_d[:])
            wk1t_sb = consts.tile([128, 2, 128], F16, tag="wk1t")
            nc.sync.dma_start(wk1t_sb[:], wk1t_d[:])
            wpool_sb = consts.tile([128, 6, 128], F16, tag="wpool")
            nc.sync.dma_start(wpool_sb[:], wpool_d[:])
            wvt_sb = consts.tile([128, 2, 256], BF16, tag="wvt")
            nc.sync.dma_start(wvt_sb[:], wvt_d[:])
            ssel_sb = consts.tile([21, HW], F16, tag="ssel")
            nc.sync.dma_start(ssel_sb[:], ssel_d[:])
            ident_sb = consts.tile([128, 128], F16, tag="ident")
            nc.sync.dma_start(ident_sb[:], ident_d[:])
            bq_sb = consts.tile([128, 1], F32, tag="bq")
            nc.sync.dma_start(bq_sb[:], bq_d[:])
            bk_sb = consts.tile([128, 1], F32, tag="bk")
            nc.sync.dma_start(bk_sb[:], bk_d[:])
            bv_sb = consts.tile([128, 2, 1], F32, tag="bv")
            nc.sync.dma_start(bv_sb[:], bv_d[:])
            # block-diagonal pyramid-pool stack; the zero blocks are written
            # once and persist across iterations
            pstack_sb = consts.tile([128, 6, 21], F16, tag="pstack")
            nc.vector.memset(pstack_sb[:], 0.0)

            regs = nc.alloc_registers("itreg")
            for reg in regs:
                nc.reg_load(reg, iters_d[0:1, 0:1])
            n_it = nc.snap(regs, min_val=1, max_val=1 << 20)

            loop_kw = {}
            if _LOOP_MODE == "staggered":
                loop_kw["staggered_reset"] = True
            elif _LOOP_MODE == "hints":
                loop_kw["hint_engines"] = (mybir.EngineType.PE,
                                           mybir.EngineType.DVE,
                                           mybir.EngineType.Activation)
            with tc.For_i(0, n_it, 1, **loop_kw):
                for b in range(BPC):
                    x1f_sb = xin.tile([128, 2, HW], F32, tag="x1f")
                    nc.sync.dma_start(x1f_sb[:], x1f_d[b])
                    x1h_sb = xin.tile([128, 2, HW], F16, tag="x1h")
                    nc.sync.dma_start(x1h_sb[:], x1h_d[b])
                    x2h_sb = xin.tile([128, 2, HW], F16, tag="x2h")
                    nc.sync.dma_start(x2h_sb[:], x2h_d[b])
                    x2b_sb = xin.tile([128, 2, HW], BF16, tag="x2b")
                    nc.sync.dma_start(x2b_sb[:], x2b_d[b])

                    if _PHASE == "dma":
                        nc.sync.dma_start(out_d[b], x1f_sb[:])
                        continue
                    # ---- pyramid pools (raw sums; scales live in ssel) ----
                    # t1[c, bw*32+h] = sum_w8 x2[c, h, bw*8+w8]
                    t1 = small.tile([128, 2, 128], F16, tag="t1")
                    for ct in range(2):
                        src = x2h_sb[:, ct, :].rearrange(
                            "p (h bw w) -> p h bw w", h=32, bw=4, w=8)
                        dst = t1[:, ct, :].rearrange("p (bw h) -> p h bw", bw=4, h=32)
                        nc.vector.tensor_reduce(
                            dst, src, axis=mybir.AxisListType.X, op=ADD)
                        # p4[c, bw*4+bh] -> pstack tile 4+ct, cols 5:21
                        src2 = t1[:, ct, :].rearrange(
                            "p (bw bh h) -> p bw bh h", bw=4, bh=4, h=8)
                        nc.vector.tensor_reduce(
                            pstack_sb[:, 4 + ct, 5:21], src2,
                            axis=mybir.AxisListType.X, op=ADD)
                        # p2[c, Bw*2+Bh] -> pstack tile 2+ct, cols 1:5
                        src3 = pstack_sb[:, 4 + ct, 5:21].rearrange(
                            "p (Bw bw Bh bh) -> p Bw Bh bw bh", Bw=2, bw=2, Bh=2, bh=2)
                        nc.vector.tensor_reduce(
                            pstack_sb[:, 2 + ct, 1:5], src3,
                            axis=mybir.AxisListType.XY, op=ADD)
                        # p1 -> pstack tile ct, col 0
                        nc.vector.tensor_reduce(
                            pstack_sb[:, ct, 0:1], pstack_sb[:, 2 + ct, 1:5],
                            axis=mybir.AxisListType.X, op=ADD)

                    # ---- q projection ----
                    q_sb = proj.tile([128, HW], F16, tag="q")
                    for ic in range(2):
                        q_ps = mm.tile([128, 512], F32, tag="mm")
                        for ct in range(2):
                            nc.tensor.matmul(
                                q_ps[:], wqt_sb[:, ct, :],
                                x1h_sb[:, ct, ds(ic * 512, 512)],
                                start=(ct == 0), stop=(ct == 1))
                        nc.vector.tensor_scalar_add(
                            q_sb[:, ds(ic * 512, 512)], q_ps[:], bq_sb[:, 0:1])

                    # ---- vT (+ ones column for the softmax denominator) ----
                    vt_sb = vtp.tile([128, 8, 257], BF16, tag="vt")
                    nc.gpsimd.memset(vt_sb[:, :, 256:257], 1.0)
                    for jp in range(4):
                        v_ps = vtps.tile([128, 2, 256], F32, tag="vtps")
                        for q_ in range(2):
                            jt = jp * 2 + q_
                            for ct in range(2):
                                nc.tensor.matmul(
                                    v_ps[:, q_, :],
                                    x2b_sb[:, ct, ds(jt * 128, 128)],
                                    wvt_sb[:, ct, :],
                                    start=(ct == 0), stop=(ct == 1))
                        for q_ in range(2):
                            nc.scalar.copy(
                                vt_sb[:, jp * 2 + q_, 0:256], v_ps[:, q_, :])

                    # ---- pyramid-pool projection m^T[21, 128] ----
                    ms_ps = mm.tile([21, 128], F32, tag="mm")
                    for t6 in range(6):
                        nc.tensor.matmul(
                            ms_ps[:], pstack_sb[:, t6, :], wpool_sb[:, t6, :],
                            start=(t6 == 0), stop=(t6 == 5))
                    ms_sb = small.tile([21, 128], F16, tag="ms")
                    nc.vector.tensor_copy(ms_sb[:], ms_ps[:])

                    # ---- k projection (x2 term + selector-matmul pool term) ----
                    k_sb = proj.tile([128, HW], F16, tag="k")
                    for ic in range(2):
                        k_ps = mm.tile([128, 512], F32, tag="mm")
                        for ct in range(2):
                            nc.tensor.matmul(
                                k_ps[:], wk1t_sb[:, ct, :],
                                x2h_sb[:, ct, ds(ic * 512, 512)],
                                start=(ct == 0), stop=False)
                        nc.tensor.matmul(
                            k_ps[:], ms_sb[:], ssel_sb[:, ds(ic * 512, 512)],
                            start=False, stop=True)
                        nc.vector.tensor_scalar_add(
                            k_sb[:, ds(ic * 512, 512)], k_ps[:], bk_sb[:, 0:1])

                    if _PHASE == "proj":
                        nc.sync.dma_start(out_d[b], x1f_sb[:])
                        continue
                    # ---- S^T = k^T q ; P^T = exp(S^T) ----
                    pt_sb = ptp.tile([128, 8, HW], BF16, tag="pt")
                    for jt in range(8):
                        for ic in range(2):
                            st_ps = mm.tile([128, 512], F32, tag="mm")
                            nc.tensor.matmul(
                                st_ps[:], k_sb[:, ds(jt * 128, 128)],
                                q_sb[:, ds(ic * 512, 512)])
                            nc.scalar.activation(
                                pt_sb[:, jt, ds(ic * 512, 512)], st_ps[:], EXP)

                    if _PHASE == "attn":
                        nc.sync.dma_start(out_d[b], x1f_sb[:])
                        continue
                    # ---- outT = P^T^T @ vT_ext ; normalize; transpose back ----
                    out_sb = outp.tile([128, 2, HW], F32, tag="osb")
                    for it in range(8):
                        o_ps = ops_.tile([128, 257], F32, tag="ops")
                        for jt in range(8):
                            nc.tensor.matmul(
                                o_ps[:], pt_sb[:, jt, ds(it * 128, 128)],
                                vt_sb[:, jt, :],
                                start=(jt == 0), stop=(jt == 7))
                        rec = recipp.tile([128, 1], F32, tag="rec")
                        nc.vector.reciprocal(rec[:], o_ps[:, 256:257])
                        on_sb = onp.tile([128, 256], F16, tag="on")
                        nc.vector.tensor_scalar_mul(
                            on_sb[:], o_ps[:, 0:256], rec[:, 0:1])
                        t_ps = vtps.tile([128, 2, 128], F16, tag="vtps")
                        for ct in range(2):
                            nc.tensor.transpose(
                                t_ps[:, ct, :], on_sb[:, ds(ct * 128, 128)],
                                ident_sb[:])
                            nc.vector.scalar_tensor_tensor(
                                out_sb[:, ct, ds(it * 128, 128)],
                                t_ps[:, ct, :], bv_sb[:, ct, 0:1],
                                x1f_sb[:, ct, ds(it * 128, 128)],
                                op0=ADD, op1=ADD)
                    nc.sync.dma_start(out_d[b], out_sb[:])

    _split_excess_waits(nc)
    return nc


# ---------------------------------------------------------------- host packing
def _pack_per_core(x1, x2):
    """[32,256,32,32] -> per-core [BPC,128,2,HW] arrays (f32/f16/f16/bf16)."""
    bf16 = ml_dtypes.bfloat16
    per_core = []
    for c in range(NCORES):
        x1c = np.ascontiguousarray(
            x1[c * BPC:(c + 1) * BPC].reshape(BPC, 2, 128, HW).transpose(0, 2, 1, 3))
        x2c = np.ascontiguousarray(
            x2[c * BPC:(c + 1) * BPC].reshape(BPC, 2, 128, HW).transpose(0, 2, 1, 3))
        per_core.append({
            "x1f": x1c.astype(np.float32),
            "x1h": x1c.astype(np.float16),
            "x2h": x2c.astype(np.float16),
            "x2b": x2c.astype(bf16),
        })
    return per_core


def _pack_weights(Wq, bq, Wk, bk, Wv, bv):
    bf16 = ml_dtypes.bfloat16
    wqt = np.ascontiguousarray(
        Wq.T.reshape(2, 128, 128).transpose(1, 0, 2)).astype(np.float16)
    wk1t = np.ascontiguousarray(
        Wk[:, 0:256].T.reshape(2, 128, 128).transpose(1, 0, 2)).astype(np.float16)
    wpool = np.ascontiguousarray(
        np.concatenate([Wk[:, 256:512].T, Wk[:, 512:768].T, Wk[:, 768:1024].T], axis=0)
        .reshape(6, 128, 128).transpose(1, 0, 2)).astype(np.float16)
    wvt = np.ascontiguousarray(
        Wv.T.reshape(2, 128, 256).transpose(1, 0, 2)).astype(bf16)

    hh, ww = np.meshgrid(np.arange(32), np.arange(32), indexing="ij")
    hh, ww = hh.ravel(), ww.ravel()
    S = np.zeros((21, HW), np.float32)
    S[0, :] = 1.0 / 1024
    for Bh in range(2):
        for Bw in range(2):
            S[1 + Bw * 2 + Bh, (hh // 16 == Bh) & (ww // 16 == Bw)] = 1.0 / 256
    for bh in range(4):
        for bw in range(4):
            S[5 + bw * 4 + bh, (hh // 8 == bh) & (ww // 8 == bw)] = 1.0 / 64

    return {
        "wqt": wqt, "wk1t": wk1t, "wpool": wpool, "wvt": wvt,
        "ssel": S.astype(np.float16),
        "ident": np.eye(128, dtype=np.float16),
        "bq": np.asarray(bq, np.float32).reshape(128, 1),
        "bk": np.asarray(bk, np.float32).reshape(128, 1),
        "bv": np.ascontiguousarray(
            np.asarray(bv, np.float32).reshape(2, 128, 1).transpose(1, 0, 2)),
    }


# ---------------------------------------------------------------- executor
class _SpmdRunner:
    """Builds the jitted 8-core callable once; repeated calls are cheap."""

    def __init__(self, nc):
        install_neuronx_cc_hook()
        self.nc = nc
        partition_name = nc.partition_id_tensor.name if nc.partition_id_tensor else None

        in_names, out_names, out_avals = [], [], []
        for alloc in nc.m.functions[0].allocations:
            if not isinstance(alloc, mybir.MemoryLocationSet):
                continue
            name = alloc.memorylocations[0].name
            if alloc.kind == "ExternalInput":
                if name != partition_name:
                    in_names.append(name)
            elif alloc.kind == "ExternalOutput":
                out_names.append(name)
                out_avals.append(jax.core.ShapedArray(
                    tuple(alloc.tensor_shape), mybir.dt.np(alloc.dtype)))
        self.in_names, self.out_names, self.out_avals = in_names, out_names, out_avals
        n_params, n_outs = len(in_names), len(out_names)
        self.n_params = n_params
        all_in_names = list(in_names) + list(out_names)
        if partition_name is not None:
            all_in_names.append(partition_name)

        def _body(*args):
            operands = list(args)
            if partition_name is not None:
                operands.append(partition_id_tensor())
            return tuple(_bass_exec_p.bind(
                *operands,
                out_avals=tuple(out_avals),
                in_names=tuple(all_in_names),
                out_names=tuple(out_names),
                lowering_input_output_aliases=(),
                sim_require_finite=True,
                sim_require_nnan=True,
                nc=nc,
            ))

        devices = jax.devices()[:NCORES]
        self.mesh = Mesh(np.asarray(devices), ("core",))
        self.fn = jax.jit(
            shard_map(_body, mesh=self.mesh,
                      in_specs=(PartitionSpec("core"),) * (n_params + n_outs),
                      out_specs=(PartitionSpec("core"),) * n_outs,
                      check_rep=False),
            keep_unused=True,
        )
        self._dev_args = None

    def put_inputs(self, in_maps):
        per_core = [[np.asarray(m[name]) for name in self.in_names] for m in in_maps]
        concat_in = [
            np.concatenate([per_core[c][i] for c in range(NCORES)], axis=0)
            for i in range(self.n_params)
        ]
        concat_zeros = [
            np.zeros((NCORES * a.shape[0], *a.shape[1:]), a.dtype)
            for a in self.out_avals
        ]
        sharding = jax.sharding.NamedSharding(self.mesh, PartitionSpec("core"))
        self._dev_args = [jax.device_put(a, sharding)
                          for a in (*concat_in, *concat_zeros)]

    def run(self):
        outs = self.fn(*self._dev_args)
        jax.block_until_ready(outs)
        return outs

    def results(self, outs):
        return [
            {name: np.asarray(outs[i]).reshape(NCORES, *self.out_avals[i].shape)[c]
             for i, name in enumerate(self.out_names)}
            for c in range(NCORES)
        ]


_RUNNER = None


def _get_runner():
    global _RUNNER
    if _RUNNER is None:
        _RUNNER = _SpmdRunner(build_nc())
    return _RUNNER


def _make_in_maps(inputs, iters):
    x1 = np.asarray(inputs["x1"], np.float32)
    x2 = np.asarray(inputs["x2"], np.float32)
    weights = _pack_weights(
        np.asarray(inputs["Wq"], np.float32), np.asarray(inputs["bq"], np.float32),
        np.asarray(inputs["Wk"], np.float32), np.asarray(inputs["bk"], np.float32),
        np.asarray(inputs["Wv"], np.float32), np.asarray(inputs["bv"], np.float32))
    per_core = _pack_per_core(x1, x2)
    it_arr = np.array([[iters]], np.int32)
    return [{**pc, **weights, "iters": it_arr} for pc in per_core]


def kernel(**inputs) -> np.ndarray:
    runner = _get_runner()
    runner.put_inputs(_make_in_maps(inputs, iters=1))
    res = runner.results(runner.run())
    out = np.empty((NCORES * BPC, 256, 32, 32), np.float32)
    for c in range(NCORES):
        oc = res[c]["out"]                       # [BPC, 128, 2, HW]
        out[c * BPC:(c + 1) * BPC] = (
            oc.transpose(0, 2, 1, 3).reshape(BPC, 256, 32, 32))
    return out


def benchmark(inputs, r_lo=1, r_hi=65, n_timing=12):
    """Per-iteration HW time in ns, via an in-kernel repeat loop: the same
    NEFF runs with iters=r_lo and iters=r_hi; slope removes dispatch cost."""
    import time
    runner = _get_runner()
    t = {}
    for R in (r_lo, r_hi):
        runner.put_inputs(_make_in_maps(inputs, iters=R))
        runner.run()
        samples = []
        for _ in range(n_timing):
            t0 = time.perf_counter()
            runner.run()
            samples.append(time.perf_counter() - t0)
        t[R] = min(samples)
    return (t[r_hi] - t[r_lo]) / (r_hi - r_lo) * 1e9


# revision 5
# speedup vs baseline: 2.7707x; 2.7707x over previous
"""Trainium2 Bass kernel for nn_CrossAttention (B=32, C=256, H=W=32).

Data-parallel over 8 NeuronCores: core c processes batches 4c..4c+4.

Per-batch dataflow on one core (channel dim on partitions, HW on free):
  q  = Wq @ x1 + bq                      (fp16 matmuls, fp32 psum)
  k  = Wk1 @ x2 + (pool-term) + bk       pool-term via a [21 x HW] selector
                                         matmul (pyramid pools never upsampled)
  vT = x2^T @ Wv^T (+ ones column)       (bf16)
  S^T = k^T q                            computed directly in [j, i] layout
  P^T = exp(S^T)                         (bf16; no max-subtraction -- logits
                                          bounded, verified on actual data)
  outT[i, 257] = sum_j P^T[j,i] vT_ext[j,:]   -> col 256 = softmax denominator
  out = transpose(outT * 1/denom) + bv + x1   (PE transpose, fused DVE epilogue)
"""
import numpy as np
import ml_dtypes

import jax
from jax.sharding import Mesh, PartitionSpec
from jax.experimental.shard_map import shard_map

import concourse.bass as bass
import concourse.mybir as mybir
import concourse.tile as tile
from concourse.bass import ds
from concourse.bass2jax import _bass_exec_p, install_neuronx_cc_hook, partition_id_tensor

F32, F16, BF16, I32 = (mybir.dt.float32, mybir.dt.float16,
                       mybir.dt.bfloat16, mybir.dt.int32)
NCORES = 8
BPC = 4          # batches per core
HW = 1024
EXP = mybir.ActivationFunctionType.Exp
ADD = mybir.AluOpType.add
import os as _os
_PHASE = _os.environ.get("KERNEL_PHASE", "full")   # dma|proj|attn|full
_LOOP_MODE = _os.environ.get("KERNEL_LOOP", "plain")  # plain|staggered|hints


# ---------------------------------------------------------------- toolchain fix
def _split_excess_waits(nc, max_waits=1):
    """This walrus build rejects >1 sem wait per instruction ("Too many sync
    wait commands"); move excess waits onto preceding same-engine NOPs (the
    sequencer executes them in order, so semantics are preserved)."""
    n_split = 0
    for f in nc.m.functions:
        for bb in f.blocks:
            idx = 0
            while idx < len(bb.instructions):
                inst = bb.instructions[idx]
                si = inst.sync_info
                if si is not None and si.on_wait and len(si.on_wait) > max_waits:
                    waits = list(si.on_wait)
                    extra, keep = waits[:-max_waits], waits[-max_waits:]
                    pos = idx
                    for j in range(0, len(extra), max_waits):
                        chunk = extra[j:j + max_waits]
                        nop = mybir.InstNoOp(name=f"waitsplit-{n_split}", ins=[], outs=[])
                        n_split += 1
                        nop.engine = inst.engine
                        nop.sync_info = mybir.SyncInfo(on_wait=chunk, on_update=[])
                        nc.register_instruction(nop, overwrite=True)
                        bb.instructions.insert(pos, nop)
                        pos += 1
                    inst.sync_info = mybir.SyncInfo(
                        on_wait=keep, on_update=list(si.on_update or []))
                    idx = pos + 1
                else:
                    idx += 1
    return n_split


# ---------------------------------------------------------------- bass builder
def build_nc():
    nc = bass.Bass("TRN2")

    x1f_d = nc.dram_tensor("x1f", [BPC, 128, 2, HW], F32, kind="ExternalInput")
    x1h_d = nc.dram_tensor("x1h", [BPC, 128, 2, HW], F16, kind="ExternalInput")
    x2h_d = nc.dram_tensor("x2h", [BPC, 128, 2, HW], F16, kind="ExternalInput")
    x2b_d = nc.dram_tensor("x2b", [BPC, 128, 2, HW], BF16, kind="ExternalInput")
    wqt_d = nc.dram_tensor("wqt", [128, 2, 128], F16, kind="ExternalInput")
    wk1t_d = nc.dram_tensor("wk1t", [128, 2, 128], F16, kind="ExternalInput")
    wpool_d = nc.dram_tensor("wpool", [128, 6, 128], F16, kind="ExternalInput")
    wvt_d = nc.dram_tensor("wvt", [128, 2, 256], BF16, kind="ExternalInput")
    ssel_d = nc.dram_tensor("ssel", [21, HW], F16, kind="ExternalInput")
    ident_d = nc.dram_tensor("ident", [128, 128], F16, kind="ExternalInput")
    bq_d = nc.dram_tensor("bq", [128, 1], F32, kind="ExternalInput")
    bk_d = nc.dram_tensor("bk", [128, 1], F32, kind="ExternalInput")
    bv_d = nc.dram_tensor("bv", [128, 2, 1], F32, kind="ExternalInput")
    iters_d = nc.dram_tensor("iters", [1, 1], I32, kind="ExternalInput")
    out_d = nc.dram_tensor("out", [BPC, 128, 2, HW], F32, kind="ExternalOutput")

    with tile.TileContext(nc) as tc:
        with (
            tc.tile_pool(name="consts", bufs=1) as consts,
            tc.tile_pool(name="xin", bufs=2) as xin,
            tc.tile_pool(name="proj", bufs=2) as proj,
            tc.tile_pool(name="vtp", bufs=2) as vtp,
            tc.tile_pool(name="ptp", bufs=2) as ptp,
            tc.tile_pool(name="small", bufs=2) as small,
            tc.tile_pool(name="outp", bufs=2) as outp,
            tc.tile_pool(name="recipp", bufs=4) as recipp,
            tc.tile_pool(name="onp", bufs=3) as onp,
            tc.tile_pool(name="mm", bufs=2, space="PSUM") as mm,
            tc.tile_pool(name="vtps", bufs=2, space="PSUM") as vtps,
            tc.tile_pool(name="ops", bufs=4, space="PSUM") as ops_,
            nc.allow_low_precision("f16/bf16 intermediates by design"),
        ):
            wqt_sb = consts.tile([128, 2, 128], F16, tag="wqt")
            nc.sync.dma_start(wqt_sb[:], wqt_d[:])
            wk1t_sb = consts.tile([128, 2, 128], F16, tag="wk1t")
            nc.sync.dma_start(wk1t_sb[:], wk1t_d[:])
            wpool_sb = consts.tile([128, 6, 128], F16, tag="wpool")
            nc.sync.dma_start(wpool_sb[:], wpool_d[:])
            wvt_sb = consts.tile([128, 2, 256], BF16, tag="wvt")
            nc.sync.dma_start(wvt_sb[:], wvt_d[:])
            ssel_sb = consts.tile([21, HW], F16, tag="ssel")
            nc.sync.dma_start(ssel_sb[:], ssel_d[:])
            ident_sb = consts.tile([128, 128], F16, tag="ident")
            nc.sync.dma_start(ident_sb[:], ident_d[:])
            bq_sb = consts.tile([128, 1], F32, tag="bq")
            nc.sync.dma_start(bq_sb[:], bq_d[:])
            bk_sb = consts.tile([128, 1], F32, tag="bk")
            nc.sync.dma_start(bk_sb[:], bk_d[:])
            bv_sb = consts.tile([128, 2, 1], F32, tag="bv")
            nc.sync.dma_start(bv_sb[:], bv_d[:])
            # block-diagonal pyramid-pool stack; the zero blocks are written
            # once and persist across iterations
            pstack_sb = consts.tile([128, 6, 21], F16, tag="pstack")
            nc.vector.memset(pstack_sb[:], 0.0)

            regs = nc.alloc_registers("itreg")
            for reg in regs:
                nc.reg_load(reg, iters_d[0:1, 0:1])
            n_it = nc.snap(regs, min_val=1, max_val=1 << 20)

            loop_kw = {}
            if _LOOP_MODE == "staggered":
                loop_kw["staggered_reset"] = True
            elif _LOOP_MODE == "hints":
                loop_kw["hint_engines"] = (mybir.EngineType.PE,
                                           mybir.EngineType.DVE,
                                           mybir.EngineType.Activation)
            with tc.For_i(0, n_it, 1, **loop_kw):
                for b in range(BPC):
                    x1f_sb = xin.tile([128, 2, HW], F32, tag="x1f")
                    nc.sync.dma_start(x1f_sb[:], x1f_d[b])
                    x1h_sb = xin.tile([128, 2, HW], F16, tag="x1h")
                    nc.sync.dma_start(x1h_sb[:], x1h_d[b])
                    x2h_sb = xin.tile([128, 2, HW], F16, tag="x2h")
                    nc.sync.dma_start(x2h_sb[:], x2h_d[b])
                    x2b_sb = xin.tile([128, 2, HW], BF16, tag="x2b")
                    nc.sync.dma_start(x2b_sb[:], x2b_d[b])

                    if _PHASE == "dma":
                        nc.sync.dma_start(out_d[b], x1f_sb[:])
                        continue
                    # ---- pyramid pools (raw sums; scales live in ssel) ----
                    # t1[c, bw*32+h] = sum_w8 x2[c, h, bw*8+w8]
                    t1 = small.tile([128, 2, 128], F16, tag="t1")
                    for ct in range(2):
                        src = x2h_sb[:, ct, :].rearrange(
                            "p (h bw w) -> p h bw w", h=32, bw=4, w=8)
                        dst = t1[:, ct, :].rearrange("p (bw h) -> p h bw", bw=4, h=32)
                        nc.vector.tensor_reduce(
                            dst, src, axis=mybir.AxisListType.X, op=ADD)
                        # p4[c, bw*4+bh] -> pstack tile 4+ct, cols 5:21
                        src2 = t1[:, ct, :].rearrange(
                            "p (bw bh h) -> p bw bh h", bw=4, bh=4, h=8)
                        nc.vector.tensor_reduce(
                            pstack_sb[:, 4 + ct, 5:21], src2,
                            axis=mybir.AxisListType.X, op=ADD)
                        # p2[c, Bw*2+Bh] -> pstack tile 2+ct, cols 1:5
                        src3 = pstack_sb[:, 4 + ct, 5:21].rearrange(
                            "p (Bw bw Bh bh) -> p Bw Bh bw bh", Bw=2, bw=2, Bh=2, bh=2)
                        nc.vector.tensor_reduce(
                            pstack_sb[:, 2 + ct, 1:5], src3,
                            axis=mybir.AxisListType.XY, op=ADD)
                        # p1 -> pstack tile ct, col 0
                        nc.vector.tensor_reduce(
                            pstack_sb[:, ct, 0:1], pstack_sb[:, 2 + ct, 1:5],
                            axis=mybir.AxisListType.X, op=ADD)

                    # ---- q projection ----
                    q_sb = proj.tile([128, HW], F16, tag="q")
                    for ic in range(2):
                        q_ps = mm.tile([128, 512], F32, tag="mm")
                        for ct in range(2):
                            nc.tensor.matmul(
                                q_ps[:], wqt_sb[:, ct, :],
                                x1h_sb[:, ct, ds(ic * 512, 512)],
                                start=(ct == 0), stop=(ct == 1))
                        nc.vector.tensor_scalar_add(
                            q_sb[:, ds(ic * 512, 512)], q_ps[:], bq_sb[:, 0:1])

                    # ---- vT (+ ones column for the softmax denominator) ----
                    vt_sb = vtp.tile([128, 8, 257], BF16, tag="vt")
                    nc.gpsimd.memset(vt_sb[:, :, 256:257], 1.0)
                    for jp in range(4):
                        v_ps = vtps.tile([128, 2, 256], F32, tag="vtps")
                        for q_ in range(2):
                            jt = jp * 2 + q_
                            for ct in range(2):
                                nc.tensor.matmul(
                                    v_ps[:, q_, :],
                                    x2b_sb[:, ct, ds(jt * 128, 128)],
                                    wvt_sb[:, ct, :],
                                    start=(ct == 0), stop=(ct == 1))
                        for q_ in range(2):
                            nc.vector.tensor_copy(
                                vt_sb[:, jp * 2 + q_, 0:256], v_ps[:, q_, :])

                    # ---- pyramid-pool projection m^T[21, 128] ----
                    ms_ps = mm.tile([21, 128], F32, tag="mm")
                    for t6 in range(6):
                        nc.tensor.matmul(
                            ms_ps[:], pstack_sb[:, t6, :], wpool_sb[:, t6, :],
                            start=(t6 == 0), stop=(t6 == 5))
                    ms_sb = small.tile([21, 128], F16, tag="ms")
                    nc.vector.tensor_copy(ms_sb[:], ms_ps[:])

                    # ---- k projection (x2 term + selector-matmul pool term) ----
                    k_sb = proj.tile([128, HW], F16, tag="k")
                    for ic in range(2):
                        k_ps = mm.tile([128, 512], F32, tag="mm")
                        for ct in range(2):
                            nc.tensor.matmul(
                                k_ps[:], wk1t_sb[:, ct, :],
                                x2h_sb[:, ct, ds(ic * 512, 512)],
                                start=(ct == 0), stop=False)
                        nc.tensor.matmul(
                            k_ps[:], ms_sb[:], ssel_sb[:, ds(ic * 512, 512)],
                            start=False, stop=True)
                        nc.vector.tensor_scalar_add(
                            k_sb[:, ds(ic * 512, 512)], k_ps[:], bk_sb[:, 0:1])

                    if _PHASE == "proj":
                        nc.sync.dma_start(out_d[b], x1f_sb[:])
                        continue
                    # ---- S^T = k^T q ; P^T = exp(S^T) ----
                    pt_sb = ptp.tile([128, 8, HW], BF16, tag="pt")
                    for jt in range(8):
                        for ic in range(2):
                            st_ps = mm.tile([128, 512], F32, tag="mm")
                            nc.tensor.matmul(
                                st_ps[:], k_sb[:, ds(jt * 128, 128)],
                                q_sb[:, ds(ic * 512, 512)])
                            nc.scalar.activation(
                                pt_sb[:, jt, ds(ic * 512, 512)], st_ps[:], EXP)

                    if _PHASE == "attn":
                        nc.sync.dma_start(out_d[b], x1f_sb[:])
                        continue
                    # ---- outT = P^T^T @ vT_ext ; normalize; transpose back ----
                    out_sb = outp.tile([128, 2, HW], F32, tag="osb")
                    for it in range(8):
                        o_ps = ops_.tile([128, 257], F32, tag="ops")
                        for jt in range(8):
                            nc.tensor.matmul(
                                o_ps[:], pt_sb[:, jt, ds(it * 128, 128)],
                                vt_sb[:, jt, :],
                                start=(jt == 0), stop=(jt == 7))
                        rec = recipp.tile([128, 1], F32, tag="rec")
                        nc.vector.reciprocal(rec[:], o_ps[:, 256:257])
                        on_sb = onp.tile([128, 256], F16, tag="on")
                        nc.vector.tensor_scalar_mul(
                            on_sb[:], o_ps[:, 0:256], rec[:, 0:1])
                        t_ps = vtps.tile([128, 2, 128], F16, tag="vtps")
                        for ct in range(2):
                            nc.tensor.transpose(
                                t_ps[:, ct, :], on_sb[:, ds(ct * 128, 128)],
                                ident_sb[:])
                            nc.vector.scalar_tensor_tensor(
                                out_sb[:, ct, ds(it * 128, 128)],
                                t_ps[:, ct, :], bv_sb[:, ct, 0:1],
                                x1f_sb[:, ct, ds(it * 128, 128)],
                                op0=ADD, op1=ADD)
                    nc.sync.dma_start(out_d[b], out_sb[:])

    _split_excess_waits(nc)
    return nc


# ---------------------------------------------------------------- host packing
def _pack_per_core(x1, x2):
    """[32,256,32,32] -> per-core [BPC,128,2,HW] arrays (f32/f16/f16/bf16)."""
    bf16 = ml_dtypes.bfloat16
    per_core = []
    for c in range(NCORES):
        x1c = np.ascontiguousarray(
            x1[c * BPC:(c + 1) * BPC].reshape(BPC, 2, 128, HW).transpose(0, 2, 1, 3))
        x2c = np.ascontiguousarray(
            x2[c * BPC:(c + 1) * BPC].reshape(BPC, 2, 128, HW).transpose(0, 2, 1, 3))
        per_core.append({
            "x1f": x1c.astype(np.float32),
            "x1h": x1c.astype(np.float16),
            "x2h": x2c.astype(np.float16),
            "x2b": x2c.astype(bf16),
        })
    return per_core


def _pack_weights(Wq, bq, Wk, bk, Wv, bv):
    bf16 = ml_dtypes.bfloat16
    wqt = np.ascontiguousarray(
        Wq.T.reshape(2, 128, 128).transpose(1, 0, 2)).astype(np.float16)
    wk1t = np.ascontiguousarray(
        Wk[:, 0:256].T.reshape(2, 128, 128).transpose(1, 0, 2)).astype(np.float16)
    wpool = np.ascontiguousarray(
        np.concatenate([Wk[:, 256:512].T, Wk[:, 512:768].T, Wk[:, 768:1024].T], axis=0)
        .reshape(6, 128, 128).transpose(1, 0, 2)).astype(np.float16)
    wvt = np.ascontiguousarray(
        Wv.T.reshape(2, 128, 256).transpose(1, 0, 2)).astype(bf16)

    hh, ww = np.meshgrid(np.arange(32), np.arange(32), indexing="ij")
    hh, ww = hh.ravel(), ww.ravel()
    S = np.zeros((21, HW), np.float32)
    S[0, :] = 1.0 / 1024
    for Bh in range(2):
        for Bw in range(2):
            S[1 + Bw * 2 + Bh, (hh // 16 == Bh) & (ww // 16 == Bw)] = 1.0 / 256
    for bh in range(4):
        for bw in range(4):
            S[5 + bw * 4 + bh, (hh // 8 == bh) & (ww // 8 == bw)] = 1.0 / 64

    return {
        "wqt": wqt, "wk1t": wk1t, "wpool": wpool, "wvt": wvt,
        "ssel": S.astype(np.float16),
        "ident": np.eye(128, dtype=np.float16),
        "bq": np.asarray(bq, np.float32).reshape(128, 1),
        "bk": np.asarray(bk, np.float32).reshape(128, 1),
        "bv": np.ascontiguousarray(
            np.asarray(bv, np.float32).reshape(2, 128, 1).transpose(1, 0, 2)),
    }


# ---------------------------------------------------------------- executor
class _SpmdRunner:
    """Builds the jitted 8-core callable once; repeated calls are cheap."""

    def __init__(self, nc):
        install_neuronx_cc_hook()
        self.nc = nc
        partition_name = nc.partition_id_tensor.name if nc.partition_id_tensor else None

        in_names, out_names, out_avals = [], [], []
        for alloc in nc.m.functions[0].allocations:
            if not isinstance(alloc, mybir.MemoryLocationSet):
                continue
            name = alloc.memorylocations[0].name
            if alloc.kind == "ExternalInput":
                if name != partition_name:
                    in_names.append(name)
            elif alloc.kind == "ExternalOutput":
                out_names.append(name)
                out_avals.append(jax.core.ShapedArray(
                    tuple(alloc.tensor_shape), mybir.dt.np(alloc.dtype)))
        self.in_names, self.out_names, self.out_avals = in_names, out_names, out_avals
        n_params, n_outs = len(in_names), len(out_names)
        self.n_params = n_params
        all_in_names = list(in_names) + list(out_names)
        if partition_name is not None:
            all_in_names.append(partition_name)

        def _body(*args):
            operands = list(args)
            if partition_name is not None:
                operands.append(partition_id_tensor())
            return tuple(_bass_exec_p.bind(
                *operands,
                out_avals=tuple(out_avals),
                in_names=tuple(all_in_names),
                out_names=tuple(out_names),
                lowering_input_output_aliases=(),
                sim_require_finite=True,
                sim_require_nnan=True,
                nc=nc,
            ))

        devices = jax.devices()[:NCORES]
        self.mesh = Mesh(np.asarray(devices), ("core",))
        self.fn = jax.jit(
            shard_map(_body, mesh=self.mesh,
                      in_specs=(PartitionSpec("core"),) * (n_params + n_outs),
                      out_specs=(PartitionSpec("core"),) * n_outs,
                      check_rep=False),
            keep_unused=True,
        )
        self._dev_args = None

    def put_inputs(self, in_maps):
        per_core = [[np.asarray(m[name]) for name in self.in_names] for m in in_maps]
        concat_in = [
            np.concatenate([per_core[c][i] for c in range(NCORES)], axis=0)
            for i in range(self.n_params)
        ]
        concat_zeros = [
            np.zeros((NCORES * a.shape[0], *a.shape[1:]), a.dtype)
            for a in self.out_avals
        ]
        sharding = jax.sharding.NamedSharding(self.mesh, PartitionSpec("core"))
        self._dev_args = [jax.device_put(a, sharding)
                          for a in (*concat_in, *concat_zeros)]

    def run(self):
        outs = self.fn(*self._dev_args)
        jax.block_until_ready(outs)
        return outs

    def results(self, outs):
        return [
            {name: np.asarray(outs[i]).reshape(NCORES, *self.out_avals[i].shape)[c]
             for i, name in enumerate(self.out_names)}
            for c in range(NCORES)
        ]


_RUNNER = None


def _get_runner():
    global _RUNNER
    if _RUNNER is None:
        _RUNNER = _SpmdRunner(build_nc())
    return _RUNNER


def _make_in_maps(inputs, iters):
    x1 = np.asarray(inputs["x1"], np.float32)
    x2 = np.asarray(inputs["x2"], np.float32)
    weights = _pack_weights(
        np.asarray(inputs["Wq"], np.float32), np.asarray(inputs["bq"], np.float32),
        np.asarray(inputs["Wk"], np.float32), np.asarray(inputs["bk"], np.float32),
        np.asarray(inputs["Wv"], np.float32), np.asarray(inputs["bv"], np.float32))
    per_core = _pack_per_core(x1, x2)
    it_arr = np.array([[iters]], np.int32)
    return [{**pc, **weights, "iters": it_arr} for pc in per_core]


def kernel(**inputs) -> np.ndarray:
    runner = _get_runner()
    runner.put_inputs(_make_in_maps(inputs, iters=1))
    res = runner.results(runner.run())
    out = np.empty((NCORES * BPC, 256, 32, 32), np.float32)
    for c in range(NCORES):
        oc = res[c]["out"]                       # [BPC, 128, 2, HW]
        out[c * BPC:(c + 1) * BPC] = (
            oc.transpose(0, 2, 1, 3).reshape(BPC, 256, 32, 32))
    return out


def benchmark(inputs, r_lo=1, r_hi=65, n_timing=12):
    """Per-iteration HW time in ns, via an in-kernel repeat loop: the same
    NEFF runs with iters=r_lo and iters=r_hi; slope removes dispatch cost."""
    import time
    runner = _get_runner()
    t = {}
    for R in (r_lo, r_hi):
        runner.put_inputs(_make_in_maps(inputs, iters=R))
        runner.run()
        samples = []
        for _ in range(n_timing):
            t0 = time.perf_counter()
            runner.run()
            samples.append(time.perf_counter() - t0)
        t[R] = min(samples)
    return (t[r_hi] - t[r_lo]) / (r_hi - r_lo) * 1e9
